# revision 8
# baseline (speedup 1.0000x reference)
"""RNN-T JointNet kernel for 8 Trainium2 NeuronCores.

Math: out[b,t,u,:] = gelu_tanh(concat(enc[b,t], dec[b,u])) @ W_fc^T + b_fc
Since gelu is elementwise, gelu(concat(a,b)) = concat(gelu(a), gelu(b)), so
  out[b,t,u,:] = P_enc[b,t,:] + P_dec[b,u,:]
with P_enc = gelu(enc) @ W_fc[:, :512]^T + b_fc  (tiny matmul, (B,T,V))
     P_dec = gelu(dec) @ W_fc[:, 512:]^T          (tiny matmul, (B,U,V))
The dominant cost is streaming the (B,T,U,V) = 310MB output to HBM.

Sharding: B*T = 1200 rows split 150 per core (core c -> b = c//2,
t in [ (c%2)*150, (c%2+1)*150 )).  Each core needs only its enc slice,
its b's dec row block, the full (pre-transposed) weight, and writes a
contiguous (150, 101, 640) output block.

Device pipeline per core:
  1. gelu(enc_slice), gelu(dec_b) on ACT engine.
  2. PE-transpose activations to [d, t] layout (identity matmul).
  3. PE matmuls -> P_enc [150,640] (bias folded in via K=1 ones matmul),
     P_dec [101,640]; copies to SBUF.
  4. Loop over t (150): PE broadcasts P_enc row across 101 partitions via
     K=1 matmul (ones[1,101]^T @ row[1,640]) into PSUM; DVE adds P_dec;
     DMA the [101, 640] tile to DRAM (contiguous 258KB block).
"""

import numpy as np

B, T, U = 4, 300, 101
D = 512
V = 640
TSLICE = 150
NCORES = 8

LAST_RESULT = None  # BassKernelResults of the most recent run (for test.py)
RUN_KWARGS = {}  # extra kwargs test.py may inject (e.g. tmpdir for traces)

_cache = {}


def _build():
    import concourse.bass as bass
    import concourse.mybir as mybir
    from concourse import bacc, masks
    from concourse.tile import TileContext

    f32 = mybir.dt.float32
    AF = mybir.ActivationFunctionType

    nc = bacc.Bacc()
    enc_d = nc.dram_tensor("enc", [TSLICE, D], f32, kind="ExternalInput")
    dec_d = nc.dram_tensor("dec", [U, D], f32, kind="ExternalInput")
    wT_d = nc.dram_tensor("wT", [2 * D, V], f32, kind="ExternalInput")
    bias_d = nc.dram_tensor("bias", [1, V], f32, kind="ExternalInput")
    out_d = nc.dram_tensor("out", [TSLICE, U, V], f32, kind="ExternalOutput")

    tchunks = [(0, 128), (128, TSLICE - 128)]
    vchunks = [(0, 512), (512, V - 512)]

    with TileContext(nc) as tc:
        with (
            tc.tile_pool(name="const", bufs=1) as constp,
            tc.tile_pool(name="work", bufs=2) as work,
            tc.tile_pool(name="persist", bufs=1) as persist,
            tc.tile_pool(name="outp", bufs=8) as outp,
            tc.tile_pool(name="rowp", bufs=2) as rowp,
            tc.tile_pool(name="dramp", bufs=1, space="DRAM") as dramp,
            tc.tile_pool(name="pre_psum", bufs=1, space="PSUM") as pre_psum,
            tc.tile_pool(name="loop_psum", bufs=2, space="PSUM") as loop_psum,
        ):
            ident = constp.tile([128, 128], f32)
            masks.make_identity(nc, ident[:])
            ones = constp.tile([1, 128], f32)
            nc.gpsimd.memset(ones[:], 1.0)
            bias_sb = constp.tile([1, V], f32)
            nc.sync.dma_start(bias_sb[:], bias_d[:])

            # dummy PE op: absorbs the gpsimd-sem wait once so later
            # transposes/matmuls never carry >1 wait (S3_LW slot limit)
            warm = pre_psum.tile([128, 128], f32, tag="tr")
            nc.tensor.transpose(warm[:32, :32], ident[:32, :32], ident[:32, :32])

            # weights: [1024 -> 8 chunks of 128 on partitions, 640 free]
            w_sb = []
            for kc in range(8):
                wt = persist.tile([128, V], f32, tag=f"w{kc}")
                nc.sync.dma_start(wt[:], wT_d[kc * 128 : (kc + 1) * 128, :])
                w_sb.append(wt)

            # gelu(enc), gelu(dec)
            genc = []
            for i, (t0, tn) in enumerate(tchunks):
                et = work.tile([128, D], f32, tag="ld_in")
                nc.sync.dma_start(et[:tn, :], enc_d[t0 : t0 + tn, :])
                gt = persist.tile([128, D], f32, tag=f"genc{i}")
                nc.scalar.activation(gt[:tn, :], et[:tn, :], AF.Gelu_apprx_tanh)
                genc.append(gt)
            dt_in = work.tile([128, D], f32, tag="ld_in")
            nc.sync.dma_start(dt_in[:U, :], dec_d[:, :])
            gdec = persist.tile([128, D], f32, tag="gdec")
            nc.scalar.activation(gdec[:U, :], dt_in[:U, :], AF.Gelu_apprx_tanh)

            # transpose to [d, t] / [d, u]
            gencT = [persist.tile([128, TSLICE], f32, tag=f"gencT{d}", name=f"gencT{d}") for d in range(4)]
            gdecT = [persist.tile([128, U], f32, tag=f"gdecT{d}", name=f"gdecT{d}") for d in range(4)]
            for dch in range(4):
                dsl = slice(dch * 128, (dch + 1) * 128)
                for i, (t0, tn) in enumerate(tchunks):
                    ps = pre_psum.tile([128, 128], f32, tag="tr")
                    nc.tensor.transpose(ps[:, :tn], genc[i][:tn, dsl], ident[:tn, :tn])
                    nc.scalar.copy(gencT[dch][:, t0 : t0 + tn], ps[:, :tn])
                ps = pre_psum.tile([128, 128], f32, tag="tr")
                nc.tensor.transpose(ps[:, :U], gdec[:U, dsl], ident[:U, :U])
                nc.scalar.copy(gdecT[dch][:, :U], ps[:, :U])

            # P_enc (with bias), P_dec
            pe_sb = [persist.tile([128, V], f32, tag=f"pe{i}", name=f"pe{i}") for i in range(2)]
            pd_sb = persist.tile([128, V], f32, tag="pd")
            for i, (t0, tn) in enumerate(tchunks):
                for v0, vn in vchunks:
                    ps = pre_psum.tile([128, 512], f32, tag="mm")
                    for d in range(4):
                        nc.tensor.matmul(
                            ps[:tn, :vn],
                            gencT[d][:, t0 : t0 + tn],
                            w_sb[d][:, v0 : v0 + vn],
                            start=(d == 0),
                            stop=False,
                        )
                    nc.tensor.matmul(
                        ps[:tn, :vn],
                        ones[:1, :tn],
                        bias_sb[:1, v0 : v0 + vn],
                        start=False,
                        stop=True,
                    )
                    nc.scalar.copy(pe_sb[i][:tn, v0 : v0 + vn], ps[:tn, :vn])
            for v0, vn in vchunks:
                ps = pre_psum.tile([128, 512], f32, tag="mm")
                for d in range(4):
                    nc.tensor.matmul(
                        ps[:U, :vn],
                        gdecT[d][:, :U],
                        w_sb[4 + d][:, v0 : v0 + vn],
                        start=(d == 0),
                        stop=(d == 3),
                    )
                nc.scalar.copy(pd_sb[:U, v0 : v0 + vn], ps[:U, :vn])

            # bounce P_enc through DRAM so row t can be fetched to partition 0
            # (matmul operands must start at partition 0/32/64)
            pe_dram = dramp.tile([TSLICE, V], f32)
            for i, (t0, tn) in enumerate(tchunks):
                nc.sync.dma_start(pe_dram[t0 : t0 + tn, :], pe_sb[i][:tn, :])

            # main loop: broadcast P_enc row, add P_dec, store
            RB = 10  # rows per batched fetch
            for bi in range(TSLICE // RB):
                rows = rowp.tile([1, RB, V], f32, tag="rows")
                nc.sync.dma_start(rows[:1], pe_dram[bi * RB : (bi + 1) * RB, :])
                for j in range(RB):
                    t = bi * RB + j
                    ps = loop_psum.tile([128, V], f32, tag="bc")
                    for v0, vn in vchunks:
                        nc.tensor.matmul(
                            ps[:U, v0 : v0 + vn],
                            ones[:1, :U],
                            rows[:1, j, v0 : v0 + vn],
                            start=True,
                            stop=True,
                        )
                    ot = outp.tile([128, V], f32, tag="out")
                    nc.vector.tensor_add(ot[:U, :], ps[:U, :], pd_sb[:U, :])
                    nc.sync.dma_start(out_d[t], ot[:U, :])

    nc.compile()
    return nc


def kernel(encoder_outputs, decoder_outputs, W_fc, b_fc):
    global LAST_RESULT
    from concourse.bass_utils import run_bass_kernel_spmd

    enc = np.ascontiguousarray(encoder_outputs, dtype=np.float32)
    dec = np.ascontiguousarray(decoder_outputs, dtype=np.float32)
    wT = np.ascontiguousarray(np.asarray(W_fc, dtype=np.float32).T)  # (1024, 640)
    bias = np.ascontiguousarray(np.asarray(b_fc, dtype=np.float32)[None, :])

    if "nc" not in _cache:
        _cache["nc"] = _build()
    nc = _cache["nc"]

    in_maps = []
    for c in range(NCORES):
        b, th = c // 2, c % 2
        in_maps.append(
            {
                "enc": np.ascontiguousarray(enc[b, th * TSLICE : (th + 1) * TSLICE]),
                "dec": np.ascontiguousarray(dec[b]),
                "wT": wT,
                "bias": bias,
            }
        )

    import os

    res = run_bass_kernel_spmd(
        nc,
        in_maps,
        list(range(NCORES)),
        trace=bool(int(os.environ.get("KJ_TRACE", "0"))),
        **RUN_KWARGS,
    )
    LAST_RESULT = res

    out = np.empty((B, T, U, V), dtype=np.float32)
    for c in range(NCORES):
        b, th = c // 2, c % 2
        out[b, th * TSLICE : (th + 1) * TSLICE] = res.results[c]["out"]
    return out


# revision 12
# speedup vs baseline: 4.4317x; 4.4317x over previous
"""RNN-T JointNet kernel for 8 Trainium2 NeuronCores.

Math: out[b,t,u,:] = gelu_tanh(concat(enc[b,t], dec[b,u])) @ W_fc^T + b_fc
Since gelu is elementwise, gelu(concat(a,b)) = concat(gelu(a), gelu(b)), so
  out[b,t,u,:] = P_enc[b,t,:] + P_dec[b,u,:]
with P_enc = gelu(enc) @ W_fc[:, :512]^T + b_fc  (tiny matmul, (B,T,V))
     P_dec = gelu(dec) @ W_fc[:, 512:]^T          (tiny matmul, (B,U,V))
The dominant cost is streaming the (B,T,U,V) = 310MB output to HBM.

Sharding: B*T = 1200 rows split 150 per core (core c -> b = c//2,
t in [ (c%2)*150, (c%2+1)*150 )).  Each core needs only its enc slice,
its b's dec row block, the full (pre-transposed) weight, and writes a
contiguous (150, 101, 640) output block.

Device pipeline per core:
  1. gelu(enc_slice), gelu(dec_b) on ACT engine.
  2. PE-transpose activations to [d, t] layout (identity matmul).
  3. PE matmuls -> P_enc [150,640] (bias folded in via K=1 ones matmul),
     P_dec [101,640]; copies to SBUF.
  4. Loop over t (150): PE broadcasts P_enc row across 101 partitions via
     K=1 matmul (ones[1,101]^T @ row[1,640]) into PSUM; DVE adds P_dec;
     DMA the [101, 640] tile to DRAM (contiguous 258KB block).
"""

import numpy as np

B, T, U = 4, 300, 101
D = 512
V = 640
TSLICE = 150
NCORES = 8

LAST_RESULT = None  # BassKernelResults of the most recent run (for test.py)
RUN_KWARGS = {}  # extra kwargs test.py may inject (e.g. tmpdir for traces)

_cache = {}


def _build():
    import concourse.bass as bass
    import concourse.mybir as mybir
    from concourse import bacc, masks
    from concourse.tile import TileContext

    f32 = mybir.dt.float32
    AF = mybir.ActivationFunctionType

    nc = bacc.Bacc()
    enc_d = nc.dram_tensor("enc", [TSLICE, D], f32, kind="ExternalInput")
    dec_d = nc.dram_tensor("dec", [U, D], f32, kind="ExternalInput")
    wT_d = nc.dram_tensor("wT", [2 * D, V], f32, kind="ExternalInput")
    bias_d = nc.dram_tensor("bias", [1, V], f32, kind="ExternalInput")
    out_d = nc.dram_tensor("out", [TSLICE, U, V], f32, kind="ExternalOutput")

    tchunks = [(0, 128), (128, TSLICE - 128)]
    vchunks = [(0, 512), (512, V - 512)]

    with TileContext(nc) as tc:
        with (
            tc.tile_pool(name="const", bufs=1) as constp,
            tc.tile_pool(name="work", bufs=2) as work,
            tc.tile_pool(name="persist", bufs=1) as persist,
            tc.tile_pool(name="outp", bufs=3) as outp,
            tc.tile_pool(name="outp1", bufs=2) as outp1,
            tc.tile_pool(name="bcp", bufs=4) as bcp,
            tc.tile_pool(name="rowp", bufs=2) as rowp,
            tc.tile_pool(name="dramp", bufs=1, space="DRAM") as dramp,
            tc.tile_pool(name="pre_psum", bufs=1, space="PSUM") as pre_psum,
            tc.tile_pool(name="loop_psum", bufs=2, space="PSUM") as loop_psum,
        ):
            ident = constp.tile([128, 128], f32)
            masks.make_identity(nc, ident[:])
            ones = constp.tile([1, 128], f32)
            nc.gpsimd.memset(ones[:], 1.0)
            bias_sb = constp.tile([1, V], f32)
            nc.sync.dma_start(bias_sb[:], bias_d[:])

            # dummy PE op: absorbs the gpsimd-sem wait once so later
            # transposes/matmuls never carry >1 wait (S3_LW slot limit)
            warm = pre_psum.tile([128, 128], f32, tag="tr")
            nc.tensor.transpose(warm[:32, :32], ident[:32, :32], ident[:32, :32])

            # weights: [1024 -> 8 chunks of 128 on partitions, 640 free]
            w_sb = []
            for kc in range(8):
                wt = persist.tile([128, V], f32, tag=f"w{kc}")
                nc.sync.dma_start(wt[:], wT_d[kc * 128 : (kc + 1) * 128, :])
                w_sb.append(wt)

            # gelu(enc), gelu(dec)
            genc = []
            for i, (t0, tn) in enumerate(tchunks):
                et = work.tile([128, D], f32, tag="ld_in")
                nc.sync.dma_start(et[:tn, :], enc_d[t0 : t0 + tn, :])
                gt = persist.tile([128, D], f32, tag=f"genc{i}")
                nc.scalar.activation(gt[:tn, :], et[:tn, :], AF.Gelu_apprx_tanh)
                genc.append(gt)
            dt_in = work.tile([128, D], f32, tag="ld_in")
            nc.sync.dma_start(dt_in[:U, :], dec_d[:, :])
            gdec = persist.tile([128, D], f32, tag="gdec")
            nc.scalar.activation(gdec[:U, :], dt_in[:U, :], AF.Gelu_apprx_tanh)

            # transpose to [d, t] / [d, u]
            gencT = [persist.tile([128, TSLICE], f32, tag=f"gencT{d}", name=f"gencT{d}") for d in range(4)]
            gdecT = [persist.tile([128, U], f32, tag=f"gdecT{d}", name=f"gdecT{d}") for d in range(4)]
            for dch in range(4):
                dsl = slice(dch * 128, (dch + 1) * 128)
                for i, (t0, tn) in enumerate(tchunks):
                    ps = pre_psum.tile([128, 128], f32, tag="tr")
                    nc.tensor.transpose(ps[:, :tn], genc[i][:tn, dsl], ident[:tn, :tn])
                    nc.scalar.copy(gencT[dch][:, t0 : t0 + tn], ps[:, :tn])
                ps = pre_psum.tile([128, 128], f32, tag="tr")
                nc.tensor.transpose(ps[:, :U], gdec[:U, dsl], ident[:U, :U])
                nc.scalar.copy(gdecT[dch][:, :U], ps[:, :U])

            # P_enc (with bias), P_dec
            pe_sb = [persist.tile([128, V], f32, tag=f"pe{i}", name=f"pe{i}") for i in range(2)]
            pd_sb = persist.tile([128, V], f32, tag="pd")
            for i, (t0, tn) in enumerate(tchunks):
                for v0, vn in vchunks:
                    ps = pre_psum.tile([128, 512], f32, tag="mm")
                    for d in range(4):
                        nc.tensor.matmul(
                            ps[:tn, :vn],
                            gencT[d][:, t0 : t0 + tn],
                            w_sb[d][:, v0 : v0 + vn],
                            start=(d == 0),
                            stop=False,
                        )
                    nc.tensor.matmul(
                        ps[:tn, :vn],
                        ones[:1, :tn],
                        bias_sb[:1, v0 : v0 + vn],
                        start=False,
                        stop=True,
                    )
                    nc.scalar.copy(pe_sb[i][:tn, v0 : v0 + vn], ps[:tn, :vn])
            for v0, vn in vchunks:
                ps = pre_psum.tile([128, 512], f32, tag="mm")
                for d in range(4):
                    nc.tensor.matmul(
                        ps[:U, :vn],
                        gdecT[d][:, :U],
                        w_sb[4 + d][:, v0 : v0 + vn],
                        start=(d == 0),
                        stop=(d == 3),
                    )
                nc.scalar.copy(pd_sb[:U, v0 : v0 + vn], ps[:U, :vn])

            # bounce P_dec through DRAM so row u can be fetched to partition 0
            # (partition_broadcast reads the base partition of its input AP)
            pd_dram = dramp.tile([U, V], f32)
            nc.sync.dma_start(pd_dram[:, :], pd_sb[:U, :])

            # main loop over u-blocks. Per u-pair: PE broadcasts two P_dec rows
            # across partitions via K=1 matmuls into PSUM (512-aligned chunks),
            # ACT copies PSUM->SBUF, DVE adds P_enc for the 128-row t-chunk
            # (SBUF+SBUF), gpsimd adds the 22-row tail. Each block stores ~2MB
            # per HWDGE ring (sync/scalar alternate).
            UB = 6
            tn1 = TSLICE - 128  # 22
            for bi, u0 in enumerate(range(0, U, UB)):
                un = min(UB, U - u0)
                rows = rowp.tile([1, UB * V], f32, tag="rows")
                nc.sync.dma_start(rows[:1, : un * V], pd_dram[u0 : u0 + un, :])
                ot0 = outp.tile([128, UB, V], f32, tag="ot0")
                ot1 = outp1.tile([128, UB, V], f32, tag="ot1")
                for j0 in range(0, un, 2):
                    npair = min(2, un - j0)
                    fl = npair * V  # 1280 or 640 flat elems
                    ps = loop_psum.tile([128, 2 * V], f32, tag="bc")
                    for c0 in range(0, fl, 512):
                        cn = min(512, fl - c0)
                        nc.tensor.matmul(
                            ps[:, c0 : c0 + cn],
                            ones[:1, :128],
                            rows[:1, j0 * V + c0 : j0 * V + c0 + cn],
                            start=True,
                            stop=True,
                        )
                    bc = bcp.tile([128, 2 * V], f32, tag="bc_sb")
                    nc.scalar.copy(bc[:, :fl], ps[:, :fl])
                    for l in range(npair):
                        j = j0 + l
                        nc.vector.tensor_add(
                            ot0[:, j, :], pe_sb[0][:, :], bc[:, l * V : (l + 1) * V]
                        )
                        nc.gpsimd.tensor_add(
                            ot1[:tn1, j, :],
                            pe_sb[1][:tn1, :],
                            bc[:tn1, l * V : (l + 1) * V],
                        )
                e0, e1 = (nc.sync, nc.scalar) if bi % 2 == 0 else (nc.scalar, nc.sync)
                e0.dma_start(out_d[0:128, u0 : u0 + un, :], ot0[:, :un, :])
                e1.dma_start(out_d[128:TSLICE, u0 : u0 + un, :], ot1[:tn1, :un, :])

    nc.compile()
    return nc


def kernel(encoder_outputs, decoder_outputs, W_fc, b_fc):
    global LAST_RESULT
    from concourse.bass_utils import run_bass_kernel_spmd

    enc = np.ascontiguousarray(encoder_outputs, dtype=np.float32)
    dec = np.ascontiguousarray(decoder_outputs, dtype=np.float32)
    wT = np.ascontiguousarray(np.asarray(W_fc, dtype=np.float32).T)  # (1024, 640)
    bias = np.ascontiguousarray(np.asarray(b_fc, dtype=np.float32)[None, :])

    if "nc" not in _cache:
        _cache["nc"] = _build()
    nc = _cache["nc"]

    in_maps = []
    for c in range(NCORES):
        b, th = c // 2, c % 2
        in_maps.append(
            {
                "enc": np.ascontiguousarray(enc[b, th * TSLICE : (th + 1) * TSLICE]),
                "dec": np.ascontiguousarray(dec[b]),
                "wT": wT,
                "bias": bias,
            }
        )

    import os

    res = run_bass_kernel_spmd(
        nc,
        in_maps,
        list(range(NCORES)),
        trace=bool(int(os.environ.get("KJ_TRACE", "0"))),
        **RUN_KWARGS,
    )
    LAST_RESULT = res

    out = np.empty((B, T, U, V), dtype=np.float32)
    for c in range(NCORES):
        b, th = c // 2, c % 2
        out[b, th * TSLICE : (th + 1) * TSLICE] = res.results[c]["out"]
    return out


# revision 13
# speedup vs baseline: 6.6082x; 1.4911x over previous
"""RNN-T JointNet kernel for 8 Trainium2 NeuronCores.

Math: out[b,t,u,:] = gelu_tanh(concat(enc[b,t], dec[b,u])) @ W_fc^T + b_fc
Since gelu is elementwise, gelu(concat(a,b)) = concat(gelu(a), gelu(b)), so
  out[b,t,u,:] = P_enc[b,t,:] + P_dec[b,u,:]
with P_enc = gelu(enc) @ W_fc[:, :512]^T + b_fc  (tiny matmul, (B,T,V))
     P_dec = gelu(dec) @ W_fc[:, 512:]^T          (tiny matmul, (B,U,V))
The dominant cost is streaming the (B,T,U,V) = 310MB output to HBM.

Sharding: 8 cores = 4 batches x 2 u-halves. Core c -> b = c//2, u-range
[ (c%2)*52, (c%2)*52+52 ) of U padded 101->104 (pad rows are zeros and
trimmed on gather). Full T=300 per core. This halves the per-core count
of PE broadcast matmuls (the previous critical path) vs t-sharding.

Device pipeline per core:
  1. gelu(enc), gelu(dec_slice) on ACT; PE-transpose to [d, t] layout.
  2. PE matmuls -> P_enc [300,640] (bias folded via K=1 ones matmul),
     P_dec [52,640]; P_dec bounced through DRAM to a partition-0 row tile.
  3. Loop over u-pairs: PE broadcasts 2 P_dec rows across 128 partitions
     (K=1 matmuls, 512-aligned chunks into one PSUM tile), ACT copies
     PSUM->SBUF, DVE adds P_enc for t-chunks 0/1 (SBUF+SBUF), gpsimd adds
     the 44-row t-tail. Per 4-u block, 3 strided DMAs (~1.3MB) store to
     DRAM, alternating between the two HWDGE rings (sync/scalar).
"""

import numpy as np

B, T, U = 4, 300, 101
D = 512
V = 640
UCORE = 52  # u rows per core (U padded to 104)
NCORES = 8

LAST_RESULT = None  # BassKernelResults of the most recent run (for test.py)
RUN_KWARGS = {}  # extra kwargs test.py may inject (e.g. tmpdir for traces)

_cache = {}


def _build():
    import concourse.mybir as mybir
    from concourse import bacc, masks
    from concourse.tile import TileContext

    f32 = mybir.dt.float32
    AF = mybir.ActivationFunctionType

    nc = bacc.Bacc()
    enc_d = nc.dram_tensor("enc", [T, D], f32, kind="ExternalInput")
    dec_d = nc.dram_tensor("dec", [UCORE, D], f32, kind="ExternalInput")
    wT_d = nc.dram_tensor("wT", [2 * D, V], f32, kind="ExternalInput")
    bias_d = nc.dram_tensor("bias", [1, V], f32, kind="ExternalInput")
    out_d = nc.dram_tensor("out", [T, UCORE, V], f32, kind="ExternalOutput")

    tchunks = [(0, 128), (128, 128), (256, 44)]
    vchunks = [(0, 512), (512, V - 512)]

    with TileContext(nc) as tc:
        with (
            tc.tile_pool(name="const", bufs=1) as constp,
            tc.tile_pool(name="work", bufs=2) as work,
            tc.tile_pool(name="persist", bufs=1) as persist,
            tc.tile_pool(name="outp0", bufs=3) as outp0,
            tc.tile_pool(name="outp1", bufs=2) as outp1,
            tc.tile_pool(name="outp2", bufs=2) as outp2,
            tc.tile_pool(name="bcp", bufs=3) as bcp,
            tc.tile_pool(name="rowp", bufs=2) as rowp,
            tc.tile_pool(name="dramp", bufs=1, space="DRAM") as dramp,
            tc.tile_pool(name="pre_psum", bufs=1, space="PSUM") as pre_psum,
            tc.tile_pool(name="loop_psum", bufs=2, space="PSUM") as loop_psum,
        ):
            ident = constp.tile([128, 128], f32)
            masks.make_identity(nc, ident[:])
            ones = constp.tile([1, 128], f32)
            nc.gpsimd.memset(ones[:], 1.0)
            bias_sb = constp.tile([1, V], f32)
            nc.sync.dma_start(bias_sb[:], bias_d[:])

            # dummy PE op: absorbs the gpsimd-sem wait once so later
            # transposes/matmuls never carry >1 wait (S3_LW slot limit)
            warm = pre_psum.tile([128, 128], f32, tag="tr")
            nc.tensor.transpose(warm[:32, :32], ident[:32, :32], ident[:32, :32])

            # weights: [1024 -> 8 chunks of 128 on partitions, 640 free]
            w_sb = []
            for kc in range(8):
                wt = persist.tile([128, V], f32, tag=f"w{kc}", name=f"w{kc}")
                nc.sync.dma_start(wt[:], wT_d[kc * 128 : (kc + 1) * 128, :])
                w_sb.append(wt)

            # gelu(enc), gelu(dec)
            genc = []
            for i, (t0, tn) in enumerate(tchunks):
                et = work.tile([128, D], f32, tag="ld_in", name="et")
                nc.sync.dma_start(et[:tn, :], enc_d[t0 : t0 + tn, :])
                gt = persist.tile([128, D], f32, tag=f"genc{i}", name=f"genc{i}")
                nc.scalar.activation(gt[:tn, :], et[:tn, :], AF.Gelu_apprx_tanh)
                genc.append(gt)
            dt_in = work.tile([128, D], f32, tag="ld_in")
            nc.sync.dma_start(dt_in[:UCORE, :], dec_d[:, :])
            gdec = persist.tile([128, D], f32, tag="gdec")
            nc.scalar.activation(gdec[:UCORE, :], dt_in[:UCORE, :], AF.Gelu_apprx_tanh)

            # transpose to [d, t] / [d, u]
            gencT = [persist.tile([128, T], f32, tag=f"gencT{d}", name=f"gencT{d}") for d in range(4)]
            gdecT = [persist.tile([128, UCORE], f32, tag=f"gdecT{d}", name=f"gdecT{d}") for d in range(4)]
            for dch in range(4):
                dsl = slice(dch * 128, (dch + 1) * 128)
                for i, (t0, tn) in enumerate(tchunks):
                    ps = pre_psum.tile([128, 128], f32, tag="tr")
                    nc.tensor.transpose(ps[:, :tn], genc[i][:tn, dsl], ident[:tn, :tn])
                    nc.scalar.copy(gencT[dch][:, t0 : t0 + tn], ps[:, :tn])
                ps = pre_psum.tile([128, 128], f32, tag="tr")
                nc.tensor.transpose(ps[:, :UCORE], gdec[:UCORE, dsl], ident[:UCORE, :UCORE])
                nc.scalar.copy(gdecT[dch][:, :UCORE], ps[:, :UCORE])

            # P_enc (with bias), P_dec
            pe_sb = [persist.tile([128, V], f32, tag=f"pe{i}", name=f"pe{i}") for i in range(3)]
            pd_sb = persist.tile([128, V], f32, tag="pd")
            for i, (t0, tn) in enumerate(tchunks):
                for v0, vn in vchunks:
                    ps = pre_psum.tile([128, 512], f32, tag="mm")
                    for d in range(4):
                        nc.tensor.matmul(
                            ps[:tn, :vn],
                            gencT[d][:, t0 : t0 + tn],
                            w_sb[d][:, v0 : v0 + vn],
                            start=(d == 0),
                            stop=False,
                        )
                    nc.tensor.matmul(
                        ps[:tn, :vn],
                        ones[:1, :tn],
                        bias_sb[:1, v0 : v0 + vn],
                        start=False,
                        stop=True,
                    )
                    nc.scalar.copy(pe_sb[i][:tn, v0 : v0 + vn], ps[:tn, :vn])
            for v0, vn in vchunks:
                ps = pre_psum.tile([128, 512], f32, tag="mm")
                for d in range(4):
                    nc.tensor.matmul(
                        ps[:UCORE, :vn],
                        gdecT[d][:, :UCORE],
                        w_sb[4 + d][:, v0 : v0 + vn],
                        start=(d == 0),
                        stop=(d == 3),
                    )
                nc.scalar.copy(pd_sb[:UCORE, v0 : v0 + vn], ps[:UCORE, :vn])

            # bounce P_dec through DRAM so rows land on partition 0
            pd_dram = dramp.tile([UCORE, V], f32)
            nc.sync.dma_start(pd_dram[:, :], pd_sb[:UCORE, :])

            UB = 4  # u rows per store block (52 = 13 * 4)
            for bi, u0 in enumerate(range(0, UCORE, UB)):
                rows = rowp.tile([1, UB * V], f32, tag="rows")
                nc.sync.dma_start(rows[:1, :], pd_dram[u0 : u0 + UB, :])
                ots = [
                    outp0.tile([128, UB, V], f32, tag="ot0", name="ot0"),
                    outp1.tile([128, UB, V], f32, tag="ot1", name="ot1"),
                    outp2.tile([128, UB, V], f32, tag="ot2", name="ot2"),
                ]
                for j0 in (0, 2):
                    fl = 2 * V  # 1280 flat elems per pair
                    ps = loop_psum.tile([128, 2 * V], f32, tag="bc")
                    for c0 in range(0, fl, 512):
                        cn = min(512, fl - c0)
                        nc.tensor.matmul(
                            ps[:, c0 : c0 + cn],
                            ones[:1, :128],
                            rows[:1, j0 * V + c0 : j0 * V + c0 + cn],
                            start=True,
                            stop=True,
                        )
                    bc = bcp.tile([128, 2 * V], f32, tag="bc_sb")
                    nc.scalar.copy(bc[:, :], ps[:, :])
                    for l in range(2):
                        j = j0 + l
                        bcv = bc[:, l * V : (l + 1) * V]
                        nc.vector.tensor_add(ots[0][:, j, :], pe_sb[0][:, :], bcv)
                        nc.vector.tensor_add(ots[1][:, j, :], pe_sb[1][:, :], bcv)
                        nc.gpsimd.tensor_add(
                            ots[2][:44, j, :], pe_sb[2][:44, :], bc[:44, l * V : (l + 1) * V]
                        )
                engs = (
                    (nc.sync, nc.scalar, nc.sync)
                    if bi % 2 == 0
                    else (nc.scalar, nc.sync, nc.scalar)
                )
                for (t0, tn), ot, eng in zip(tchunks, ots, engs):
                    eng.dma_start(out_d[t0 : t0 + tn, u0 : u0 + UB, :], ot[:tn, :, :])

    nc.compile()
    return nc


def kernel(encoder_outputs, decoder_outputs, W_fc, b_fc):
    global LAST_RESULT
    import os

    from concourse.bass_utils import run_bass_kernel_spmd

    enc = np.ascontiguousarray(encoder_outputs, dtype=np.float32)
    dec = np.ascontiguousarray(decoder_outputs, dtype=np.float32)
    wT = np.ascontiguousarray(np.asarray(W_fc, dtype=np.float32).T)  # (1024, 640)
    bias = np.ascontiguousarray(np.asarray(b_fc, dtype=np.float32)[None, :])

    dec_pad = np.zeros((B, 2 * UCORE, D), dtype=np.float32)
    dec_pad[:, :U, :] = dec

    if "nc" not in _cache:
        _cache["nc"] = _build()
    nc = _cache["nc"]

    in_maps = []
    for c in range(NCORES):
        b, uh = c // 2, c % 2
        in_maps.append(
            {
                "enc": np.ascontiguousarray(enc[b]),
                "dec": np.ascontiguousarray(dec_pad[b, uh * UCORE : (uh + 1) * UCORE]),
                "wT": wT,
                "bias": bias,
            }
        )

    res = run_bass_kernel_spmd(
        nc,
        in_maps,
        list(range(NCORES)),
        trace=bool(int(os.environ.get("KJ_TRACE", "0"))),
        **RUN_KWARGS,
    )
    LAST_RESULT = res

    out = np.empty((B, T, U, V), dtype=np.float32)
    for c in range(NCORES):
        b, uh = c // 2, c % 2
        cut = res.results[c]["out"]  # (300, 52, 640)
        if uh == 0:
            out[b, :, :UCORE] = cut
        else:
            out[b, :, UCORE:U] = cut[:, : U - UCORE]
    return out


# revision 14
# speedup vs baseline: 6.7101x; 1.0154x over previous
"""RNN-T JointNet kernel for 8 Trainium2 NeuronCores.

Math: out[b,t,u,:] = gelu_tanh(concat(enc[b,t], dec[b,u])) @ W_fc^T + b_fc
Since gelu is elementwise, gelu(concat(a,b)) = concat(gelu(a), gelu(b)), so
  out[b,t,u,:] = P_enc[b,t,:] + P_dec[b,u,:]
with P_enc = gelu(enc) @ W_fc[:, :512]^T + b_fc  (tiny matmul, (B,T,V))
     P_dec = gelu(dec) @ W_fc[:, 512:]^T          (tiny matmul, (B,U,V))
The dominant cost is streaming the (B,T,U,V) = 310MB output to HBM.

Sharding: 8 cores = 4 batches x 2 u-halves. Core c -> b = c//2, u-range
[ (c%2)*52, (c%2)*52+52 ) of U padded 101->104 (pad rows are zeros and
trimmed on gather). Full T=300 per core. This halves the per-core count
of PE broadcast matmuls (the previous critical path) vs t-sharding.

Device pipeline per core:
  1. gelu(enc), gelu(dec_slice) on ACT; PE-transpose to [d, t] layout.
  2. PE matmuls -> P_enc [300,640] (bias folded via K=1 ones matmul),
     P_dec [52,640]; P_dec bounced through DRAM to a partition-0 row tile.
  3. Loop over u-pairs: PE broadcasts 2 P_dec rows across 128 partitions
     (K=1 matmuls, 512-aligned chunks into one PSUM tile), ACT copies
     PSUM->SBUF, DVE adds P_enc for t-chunks 0/1 (SBUF+SBUF), gpsimd adds
     the 44-row t-tail. Per 4-u block, 3 strided DMAs (~1.3MB) store to
     DRAM, alternating between the two HWDGE rings (sync/scalar).
"""

import numpy as np

B, T, U = 4, 300, 101
D = 512
V = 640
UCORE = 52  # u rows per core (U padded to 104)
NCORES = 8

LAST_RESULT = None  # BassKernelResults of the most recent run (for test.py)
RUN_KWARGS = {}  # extra kwargs test.py may inject (e.g. tmpdir for traces)

_cache = {}


def _build():
    import concourse.mybir as mybir
    from concourse import bacc, masks
    from concourse.tile import TileContext

    f32 = mybir.dt.float32
    AF = mybir.ActivationFunctionType

    nc = bacc.Bacc()
    enc_d = nc.dram_tensor("enc", [T, D], f32, kind="ExternalInput")
    dec_d = nc.dram_tensor("dec", [UCORE, D], f32, kind="ExternalInput")
    wT_d = nc.dram_tensor("wT", [2 * D, V], f32, kind="ExternalInput")
    bias_d = nc.dram_tensor("bias", [1, V], f32, kind="ExternalInput")
    out_d = nc.dram_tensor("out", [T, UCORE, V], f32, kind="ExternalOutput")

    tchunks = [(0, 128), (128, 128), (256, 44)]
    vchunks = [(0, 512), (512, V - 512)]

    with TileContext(nc) as tc:
        with (
            tc.tile_pool(name="const", bufs=1) as constp,
            tc.tile_pool(name="work", bufs=2) as work,
            tc.tile_pool(name="persist", bufs=1) as persist,
            tc.tile_pool(name="outp0", bufs=4) as outp0,
            tc.tile_pool(name="outp1", bufs=2) as outp1,
            tc.tile_pool(name="outp2", bufs=2) as outp2,
            tc.tile_pool(name="bcp", bufs=4) as bcp,
            tc.tile_pool(name="rowp", bufs=3) as rowp,
            tc.tile_pool(name="dramp", bufs=1, space="DRAM") as dramp,
            tc.tile_pool(name="pre_psum", bufs=1, space="PSUM") as pre_psum,
            tc.tile_pool(name="loop_psum", bufs=2, space="PSUM") as loop_psum,
        ):
            ident = constp.tile([128, 128], f32)
            masks.make_identity(nc, ident[:])
            ones = constp.tile([1, 128], f32)
            nc.gpsimd.memset(ones[:], 1.0)
            bias_sb = constp.tile([1, V], f32)
            nc.sync.dma_start(bias_sb[:], bias_d[:])

            # dummy PE op: absorbs the gpsimd-sem wait once so later
            # transposes/matmuls never carry >1 wait (S3_LW slot limit)
            warm = pre_psum.tile([128, 128], f32, tag="tr")
            nc.tensor.transpose(warm[:32, :32], ident[:32, :32], ident[:32, :32])

            # weights: [1024 -> 8 chunks of 128 on partitions, 640 free]
            w_sb = []
            for kc in range(8):
                wt = persist.tile([128, V], f32, tag=f"w{kc}", name=f"w{kc}")
                nc.sync.dma_start(wt[:], wT_d[kc * 128 : (kc + 1) * 128, :])
                w_sb.append(wt)

            # gelu(enc), gelu(dec)
            genc = []
            for i, (t0, tn) in enumerate(tchunks):
                et = work.tile([128, D], f32, tag="ld_in", name="et")
                nc.sync.dma_start(et[:tn, :], enc_d[t0 : t0 + tn, :])
                gt = persist.tile([128, D], f32, tag=f"genc{i}", name=f"genc{i}")
                nc.scalar.activation(gt[:tn, :], et[:tn, :], AF.Gelu_apprx_tanh)
                genc.append(gt)
            dt_in = work.tile([128, D], f32, tag="ld_in")
            nc.sync.dma_start(dt_in[:UCORE, :], dec_d[:, :])
            gdec = persist.tile([128, D], f32, tag="gdec")
            nc.scalar.activation(gdec[:UCORE, :], dt_in[:UCORE, :], AF.Gelu_apprx_tanh)

            # transpose to [d, t] / [d, u]
            gencT = [persist.tile([128, T], f32, tag=f"gencT{d}", name=f"gencT{d}") for d in range(4)]
            gdecT = [persist.tile([128, UCORE], f32, tag=f"gdecT{d}", name=f"gdecT{d}") for d in range(4)]
            for dch in range(4):
                dsl = slice(dch * 128, (dch + 1) * 128)
                for i, (t0, tn) in enumerate(tchunks):
                    ps = pre_psum.tile([128, 128], f32, tag="tr")
                    nc.tensor.transpose(ps[:, :tn], genc[i][:tn, dsl], ident[:tn, :tn])
                    nc.scalar.copy(gencT[dch][:, t0 : t0 + tn], ps[:, :tn])
                ps = pre_psum.tile([128, 128], f32, tag="tr")
                nc.tensor.transpose(ps[:, :UCORE], gdec[:UCORE, dsl], ident[:UCORE, :UCORE])
                nc.scalar.copy(gdecT[dch][:, :UCORE], ps[:, :UCORE])

            # P_enc (with bias), P_dec
            pe_sb = [persist.tile([128, V], f32, tag=f"pe{i}", name=f"pe{i}") for i in range(3)]
            pd_sb = persist.tile([128, V], f32, tag="pd")
            for i, (t0, tn) in enumerate(tchunks):
                for v0, vn in vchunks:
                    ps = pre_psum.tile([128, 512], f32, tag="mm")
                    for d in range(4):
                        nc.tensor.matmul(
                            ps[:tn, :vn],
                            gencT[d][:, t0 : t0 + tn],
                            w_sb[d][:, v0 : v0 + vn],
                            start=(d == 0),
                            stop=False,
                        )
                    nc.tensor.matmul(
                        ps[:tn, :vn],
                        ones[:1, :tn],
                        bias_sb[:1, v0 : v0 + vn],
                        start=False,
                        stop=True,
                    )
                    nc.scalar.copy(pe_sb[i][:tn, v0 : v0 + vn], ps[:tn, :vn])
            for v0, vn in vchunks:
                ps = pre_psum.tile([128, 512], f32, tag="mm")
                for d in range(4):
                    nc.tensor.matmul(
                        ps[:UCORE, :vn],
                        gdecT[d][:, :UCORE],
                        w_sb[4 + d][:, v0 : v0 + vn],
                        start=(d == 0),
                        stop=(d == 3),
                    )
                nc.scalar.copy(pd_sb[:UCORE, v0 : v0 + vn], ps[:UCORE, :vn])

            # bounce P_dec through DRAM so rows land on partition 0
            pd_dram = dramp.tile([UCORE, V], f32)
            nc.sync.dma_start(pd_dram[:, :], pd_sb[:UCORE, :])

            UB = 4  # u rows per store block (52 = 13 * 4)
            for bi, u0 in enumerate(range(0, UCORE, UB)):
                rows = rowp.tile([1, UB * V], f32, tag="rows")
                nc.sync.dma_start(rows[:1, :], pd_dram[u0 : u0 + UB, :])
                ots = [
                    outp0.tile([128, UB, V], f32, tag="ot0", name="ot0"),
                    outp1.tile([128, UB, V], f32, tag="ot1", name="ot1"),
                    outp2.tile([128, UB, V], f32, tag="ot2", name="ot2"),
                ]
                for j0 in (0, 2):
                    fl = 2 * V  # 1280 flat elems per pair
                    ps = loop_psum.tile([128, 2 * V], f32, tag="bc")
                    for c0 in range(0, fl, 512):
                        cn = min(512, fl - c0)
                        nc.tensor.matmul(
                            ps[:, c0 : c0 + cn],
                            ones[:1, :128],
                            rows[:1, j0 * V + c0 : j0 * V + c0 + cn],
                            start=True,
                            stop=True,
                        )
                    bc = bcp.tile([128, 2 * V], f32, tag="bc_sb")
                    nc.scalar.copy(bc[:, :], ps[:, :])
                    for l in range(2):
                        j = j0 + l
                        bcv = bc[:, l * V : (l + 1) * V]
                        nc.vector.tensor_add(ots[0][:, j, :], pe_sb[0][:, :], bcv)
                        nc.vector.tensor_add(ots[1][:, j, :], pe_sb[1][:, :], bcv)
                        nc.gpsimd.tensor_add(
                            ots[2][:44, j, :], pe_sb[2][:44, :], bc[:44, l * V : (l + 1) * V]
                        )
                engs = (
                    (nc.sync, nc.scalar, nc.sync)
                    if bi % 2 == 0
                    else (nc.scalar, nc.sync, nc.scalar)
                )
                for (t0, tn), ot, eng in zip(tchunks, ots, engs):
                    eng.dma_start(out_d[t0 : t0 + tn, u0 : u0 + UB, :], ot[:tn, :, :])

    nc.compile()
    return nc


def kernel(encoder_outputs, decoder_outputs, W_fc, b_fc):
    global LAST_RESULT
    import os

    from concourse.bass_utils import run_bass_kernel_spmd

    enc = np.ascontiguousarray(encoder_outputs, dtype=np.float32)
    dec = np.ascontiguousarray(decoder_outputs, dtype=np.float32)
    wT = np.ascontiguousarray(np.asarray(W_fc, dtype=np.float32).T)  # (1024, 640)
    bias = np.ascontiguousarray(np.asarray(b_fc, dtype=np.float32)[None, :])

    dec_pad = np.zeros((B, 2 * UCORE, D), dtype=np.float32)
    dec_pad[:, :U, :] = dec

    if "nc" not in _cache:
        _cache["nc"] = _build()
    nc = _cache["nc"]

    in_maps = []
    for c in range(NCORES):
        b, uh = c // 2, c % 2
        in_maps.append(
            {
                "enc": np.ascontiguousarray(enc[b]),
                "dec": np.ascontiguousarray(dec_pad[b, uh * UCORE : (uh + 1) * UCORE]),
                "wT": wT,
                "bias": bias,
            }
        )

    res = run_bass_kernel_spmd(
        nc,
        in_maps,
        list(range(NCORES)),
        trace=bool(int(os.environ.get("KJ_TRACE", "0"))),
        **RUN_KWARGS,
    )
    LAST_RESULT = res

    out = np.empty((B, T, U, V), dtype=np.float32)
    for c in range(NCORES):
        b, uh = c // 2, c % 2
        cut = res.results[c]["out"]  # (300, 52, 640)
        if uh == 0:
            out[b, :, :UCORE] = cut
        else:
            out[b, :, UCORE:U] = cut[:, : U - UCORE]
    return out
